# revision 8
# baseline (speedup 1.0000x reference)
"""Trainium2 Bass kernel for the pairwise-similarity histogram loss.

Reference computation:
  sim = x @ x.T  (rows L2-normalized), upper-tri pairs (i<j)
  soft (triangular) binning of similarities into 51 bins, separately for
  label-equal (pos) and label-unequal (neg) pairs; loss = sum(hist_neg * cumsum(hist_pos)).

Device algorithm (8 NeuronCores, SPMD, data-parallel over sim rows):
  Host sorts rows by label and hands each core a column-rotated copy of
  x_sorted.T so every core's own 128 rows sit at rotated columns 0..127 —
  all cores run an identical program.

  Per core:
    - PE: sim_shard = xT[:,0:128].T @ xT   -> PSUM [128, 1024]
    - masks from iota/label compares; s' = triu_mask * (1 + sim) in [0, 2]
    - histogramming uses the identity
        F[k] = sum_p clip((s'_p - k*bw)/bw, 0, 1) = (R[k] - R[k+1]) / bw,
        R[k] = sum_p relu(s'_p - k*bw)
      each R[k] is ONE fused instruction (relu + per-partition accumulate)
      on DVE (tensor_scalar sub/max + accum_out) or ACT (activation Relu +
      accum_out), split across engines.
    - pos pairs: after label-sorting they all live in a 64-wide diagonal
      band; the band is extracted via a skewed DRAM bounce and binned on a
      tiny [128, 63] tile.  neg = triu - pos.
  Host: f64 combine of per-partition partial sums -> final scalar loss.
"""

import numpy as np

NBINS = 51
BW = 2.0 / (NBINS - 1)
BS, D = 1024, 128
N_CORES = 8
SH = BS // N_CORES  # 128 rows per core

# ---------------- configuration ----------------
USE_BAND = False          # pos histogram via diagonal band (else dense pos passes)
KLO, KHI = 0, 51          # R[k] computed on device for k in [KLO, KHI]; outside: closed form
PASS_DT = "float32"       # dtype of the masked s' tiles the bin passes read
BANDW = 64                # band width (covers within-class pair distance <= 63)

_CACHE = {}


def _build_program():
    import concourse.bass as bass
    import concourse.bacc as bacc
    import concourse.tile as tile
    import concourse.mybir as mybir
    from concourse.ap import AP

    F32 = mybir.dt.float32
    PDT = getattr(mybir.dt, PASS_DT)
    Alu = mybir.AluOpType
    Act = mybir.ActivationFunctionType

    ks = list(range(KLO, KHI + 1))  # R[k] passes on device

    # pass plan: (family, k) -> engine + column index
    passes = []
    fams = ["tri", "pos"] if USE_BAND else ["pos", "neg"]
    for fam in fams:
        for k in ks:
            passes.append((fam, k))
    # weighted round-robin assignment DVE vs ACT by estimated per-pass cost
    est = {"D": 0.0, "A": 0.0}
    cost = {"D": 731.0, "A": 1147.0}
    plan = {}
    counts = {"D": 0, "A": 0}
    for fam, k in passes:
        if USE_BAND and fam == "pos":
            eng = "D" if (k % 2 == 0) else "A"  # tiny band passes: plain alternation
            plan[(fam, k)] = (eng, None)
            continue
        eng = "D" if est["D"] + cost["D"] <= est["A"] + cost["A"] else "A"
        est[eng] += cost[eng]
        plan[(fam, k)] = (eng, None)
    # column indexing per engine
    for key in plan:
        eng, _ = plan[key]
        plan[key] = (eng, counts[eng])
        counts[eng] += 1
    nD, nA = counts["D"], counts["A"]
    NCNT = 4  # cnt columns: cntpos, cntneg/cnttri, Spos, Stri
    NOUT = nD + nA + NCNT

    nc = bacc.Bacc("TRN2", target_bir_lowering=False, debug=False,
                   num_devices=N_CORES)

    xTrL = nc.dram_tensor("xTrL", [D, 512], F32, kind="ExternalInput")
    xTrR = nc.dram_tensor("xTrR", [D, 512], F32, kind="ExternalInput")
    collab = nc.dram_tensor("collab", [1, BS], F32, kind="ExternalInput")
    collab_bf = nc.dram_tensor("collab_bf", [1, BS], mybir.dt.bfloat16,
                               kind="ExternalInput")
    rowlab = nc.dram_tensor("rowlab", [SH, 1], F32, kind="ExternalInput")
    wrapcut = nc.dram_tensor("wrapcut", [SH, 1], F32, kind="ExternalInput")
    nk = len(ks)
    cvec = nc.dram_tensor("cvec", [SH, nk], F32, kind="ExternalInput")
    acc_out = nc.dram_tensor("acc", [SH, NOUT], F32, kind="ExternalOutput")

    with tile.TileContext(nc) as tc:
        with tc.tile_pool(name="main", bufs=1) as pool, \
             tc.tile_pool(name="psum", bufs=1, space="PSUM") as psum:
            xL = pool.tile([D, 512], F32)
            xR = pool.tile([D, 512], F32)
            nc.sync.dma_start(xL[:], xTrL[:])
            nc.sync.dma_start(xR[:], xTrR[:])

            rowlab_sb = pool.tile([SH, 1], F32)
            nc.sync.dma_start(rowlab_sb[:], rowlab[:])
            wrapcut_sb = pool.tile([SH, 1], F32)
            nc.sync.dma_start(wrapcut_sb[:], wrapcut[:])
            cvec_sb = pool.tile([SH, nk], F32)
            nc.sync.dma_start(cvec_sb[:], cvec[:])
            collab_bf_sb = pool.tile([1, BS], mybir.dt.bfloat16)
            nc.sync.dma_start(collab_bf_sb[:], collab_bf[:])

            ones_bf = pool.tile([1, D], mybir.dt.bfloat16)
            nc.vector.memset(ones_bf[:], 1.0)

            # iota along free dim (0..1023), and local row index (0..127)
            iotaT = pool.tile([SH, BS], F32)
            nc.gpsimd.iota(iotaT[:], pattern=[[1, BS]], base=0,
                           channel_multiplier=0,
                           allow_small_or_imprecise_dtypes=True)
            rowloc = pool.tile([SH, 1], F32)
            nc.gpsimd.iota(rowloc[:], pattern=[[0, 1]], base=0,
                           channel_multiplier=1,
                           allow_small_or_imprecise_dtypes=True)

            # sim = xT[:, 0:128].T @ xT  -> PSUM
            simP = psum.tile([SH, BS], F32)
            nc.tensor.matmul(simP[:, 0:512], xL[:, 0:D], xL[:])
            nc.tensor.matmul(simP[:, 512:BS], xL[:, 0:D], xR[:])

            # label broadcast via K=1 bf16 matmul (labels 0..31 exact in bf16)
            labmatP = psum.tile([SH, BS], F32)
            nc.tensor.matmul(labmatP[:, 0:512], ones_bf[:], collab_bf_sb[:, 0:512])
            nc.tensor.matmul(labmatP[:, 512:BS], ones_bf[:], collab_bf_sb[:, 512:BS])

            # triu mask (in rotated coords): (t > r) & (t < wrapcut)
            gtmask = pool.tile([SH, BS], F32)
            nc.vector.tensor_scalar(gtmask[:], iotaT[:], rowloc[:], None,
                                    op0=Alu.is_gt)
            trimask = pool.tile([SH, BS], F32)
            nc.vector.scalar_tensor_tensor(trimask[:], iotaT[:], wrapcut_sb[:],
                                           gtmask[:], op0=Alu.is_lt, op1=Alu.mult)

            # s' = 1 + sim (ACT, evacuates PSUM)
            splus = pool.tile([SH, BS], F32)
            nc.scalar.activation(splus[:], simP[:], Act.Identity, bias=1.0)

            cnts = pool.tile([SH, NCNT], F32)

            # pos/neg masks and masked s' tensors
            mpos = pool.tile([SH, BS], F32)
            nc.vector.scalar_tensor_tensor(mpos[:], labmatP[:], rowlab_sb[:],
                                           trimask[:], op0=Alu.is_equal,
                                           op1=Alu.mult,
                                           accum_out=cnts[:, 0:1])
            mneg = pool.tile([SH, BS], F32)
            nc.vector.scalar_tensor_tensor(mneg[:], trimask[:], 1.0, mpos[:],
                                           op0=Alu.mult, op1=Alu.subtract,
                                           accum_out=cnts[:, 1:2])
            spos = pool.tile([SH, BS], PDT)
            nc.vector.scalar_tensor_tensor(spos[:], mpos[:], 1.0, splus[:],
                                           op0=Alu.mult, op1=Alu.mult)
            sneg = pool.tile([SH, BS], PDT)
            nc.vector.scalar_tensor_tensor(sneg[:], mneg[:], 1.0, splus[:],
                                           op0=Alu.mult, op1=Alu.mult)
            src = {"pos": spos, "neg": sneg}

            # sums of masked s' (for closed-form low bins)
            trashD = pool.tile([SH, BS], PDT)
            trashA = pool.tile([SH, BS], PDT)
            nc.vector.tensor_scalar(trashD[:], spos[:], 1.0, 0.0, op0=Alu.mult,
                                    op1=Alu.add, accum_out=cnts[:, 2:3])
            nc.vector.tensor_scalar(trashA[:], sneg[:], 1.0, 0.0, op0=Alu.mult,
                                    op1=Alu.add, accum_out=cnts[:, 3:4])

            accD = pool.tile([SH, max(nD, 1)], F32)
            accA = pool.tile([SH, max(nA, 1)], F32)

            for fam, k in passes:
                eng, j = plan[(fam, k)]
                c = float(np.float32(k * BW))
                s_t = src[fam]
                if eng == "D":
                    # relu(s-c) = (s max c) + (-c); op1=add doubles as the
                    # accum_out reduction operator (sum)
                    nc.vector.tensor_scalar(trashD[:], s_t[:], c, -c,
                                            op0=Alu.max, op1=Alu.add,
                                            accum_out=accD[:, j:j + 1])
                else:
                    jc = k - KLO
                    nc.scalar.activation(trashA[:], s_t[:], Act.Relu,
                                         bias=cvec_sb[:, jc:jc + 1], scale=1.0,
                                         accum_out=accA[:, j:j + 1])

            nc.sync.dma_start(acc_out[:, 0:nD], accD[:])
            nc.sync.dma_start(acc_out[:, nD:nD + nA], accA[:])
            nc.sync.dma_start(acc_out[:, nD + nA:NOUT], cnts[:])

    nc.compile()
    return nc, plan, (nD, nA, NOUT)


def _get_program():
    key = (USE_BAND, KLO, KHI, PASS_DT)
    if key not in _CACHE:
        _CACHE[key] = _build_program()
    return _CACHE[key]


def _host_prep(x, labels):
    x = np.ascontiguousarray(np.asarray(x, dtype=np.float32))
    labels = np.asarray(labels).astype(np.int64)
    perm = np.argsort(labels, kind="stable")
    xs = x[perm]
    labs = labels[perm].astype(np.float32)
    xT = np.ascontiguousarray(xs.T)  # [128, 1024]
    import ml_dtypes
    in_maps = []
    for c in range(N_CORES):
        r = SH * c
        xTr = np.roll(xT, -r, axis=1)
        collab_c = np.ascontiguousarray(np.roll(labs, -r)[None, :])
        rowlab_c = np.ascontiguousarray(collab_c[0, :SH, None])
        wrapcut_c = np.full((SH, 1), float(BS - r), np.float32)
        ks_arr = np.arange(KLO, KHI + 1, dtype=np.float32)
        cvec_c = np.tile(-(ks_arr * np.float32(BW))[None, :], (SH, 1)).astype(np.float32)
        in_maps.append({
            "cvec": cvec_c,
            "xTrL": np.ascontiguousarray(xTr[:, 0:512]),
            "xTrR": np.ascontiguousarray(xTr[:, 512:]),
            "collab": collab_c,
            "collab_bf": collab_c.astype(ml_dtypes.bfloat16),
            "rowlab": rowlab_c,
            "wrapcut": wrapcut_c,
        })
    return in_maps, labels


def _combine(results, plan, meta, labels):
    nD, nA, NOUT = meta
    tot = np.zeros((NOUT,), np.float64)
    for res in results:
        tot += res["acc"].astype(np.float64).sum(axis=0)

    def col(eng, j):
        return j if eng == "D" else nD + j

    cntpos = tot[nD + nA + 0]
    cntneg = tot[nD + nA + 1]
    Spos = tot[nD + nA + 2]
    Sneg = tot[nD + nA + 3]
    npairs = BS * (BS - 1) // 2
    assert abs(cntpos + cntneg - npairs) < 0.5, (cntpos, cntneg)

    def R_of(fam, Sm, Nm):
        R = np.zeros((NBINS + 1,), np.float64)  # k = 0..51
        for k in range(NBINS + 1):
            if k < KLO:
                R[k] = Sm - Nm * (k * BW)
            elif k > KHI:
                R[k] = 0.0
            else:
                eng, j = plan[(fam, k)]
                if eng == "D":
                    # DVE pass accumulates sum(max(s,c)) - c per partition row;
                    # sum(relu(s-c)) = acc - (BS-1)*c per row, P rows total.
                    P = SH * N_CORES
                    R[k] = tot[col(eng, j)] - P * (BS - 1) * (k * BW)
                else:
                    R[k] = tot[col(eng, j)]
        return R

    Rpos = R_of("pos", Spos, cntpos)
    Rneg = R_of("neg", Sneg, cntneg)
    Fpos = (Rpos[:-1] - Rpos[1:]) / BW          # k = 0..50
    Fneg = (Rneg[:-1] - Rneg[1:]) / BW
    Fneg_m1 = cntneg
    histneg = np.empty((NBINS,), np.float64)
    histneg[0] = (Fneg_m1 - Fneg[0]) / cntneg
    histneg[1:] = (Fneg[:-1] - Fneg[1:]) / cntneg
    cdfpos = 1.0 - Fpos / cntpos
    loss = float(np.sum(histneg * cdfpos))
    return np.float32(loss)


def _run(x, labels, trace=False, trace_cores=None):
    from concourse.bass_utils import run_bass_kernel_spmd
    nc, plan, meta = _get_program()
    in_maps, labels = _host_prep(x, labels)
    out = run_bass_kernel_spmd(nc, in_maps, list(range(N_CORES)),
                               trace=trace, trace_cores=trace_cores)
    loss = _combine(out.results, plan, meta, labels)
    return loss, out


def kernel(x, labels):
    loss, _ = _run(x, labels)
    return loss
